# revision 12
# baseline (speedup 1.0000x reference)
"""Trainium2 kernel for nn_CosinePairwiseLoss.

Math: for unit-normalized rows f_i and class labels pred_i, the reference
computes   loss = 1 - mean_c [ (sum_{i<j, both in c} f_i.f_j) / C(n_c,2) ].
Since sum_{i!=j in c} f_i.f_j = ||S_c||^2 - n_c with S_c = sum_{i in c} f_i,
the strict-lower-triangle sum is (||S_c||^2 - n_c)/2.  So the whole problem
reduces to a per-class segment-sum of normalized rows (C x D) plus counts —
O(N*D) memory-bound work, no N x N similarity matrix.

Device pipeline (per core, rows sharded 8 ways, 2048 rows = 128 partitions x
16 row groups of 64 features):

  input   chunk0 (pred+first groups) via SWDGE dma_gather prep/trigger
          (descriptor prep runs on Pool right after the preamble barrier, so
          the trigger skips the 625ns HWDGE issue + 650ns DGE delay), chunk1
          via a plain HWDGE DMA on SP in parallel.
  norms   per row group: tensor_scalar(pow 2.0, accum_out) gives sum-of-
          squares in ONE 4x-mode DVE op; per chunk: tensor_scalar(pow -0.5)
          gives 1/||f||.
  onehot  per row group: tensor_scalar(iota is_equal pred_n, mult rn_n) —
          the weighted onehot (pred==c)/||f|| in one 4x-mode op, so the
          matmul consumes RAW bf16 features straight from the DMA.
  matmul  16 x (Ldweights ohw_n; Matmult f_n) accumulate S = sum ohw^T f
          into PSUM [C, D].
  output  PSUM -> SBUF copy, then a kv_writeback SWDGE DMA whose descriptors
          were prepared during the input wait; the trigger fires right after
          the copy (no HWDGE issue cost on the tail).

Odd row groups' DVE ops run on Pool (gpsimd) to halve the DVE critical path.
Host: sums the 8 partial S matrices, adds counts (bincount), finishes the
O(C) scalar math.
"""

import numpy as np

N, D, C = 16384, 64, 64
NCORES = 8
ROWS = N // NCORES  # 2048 rows per core
P = 128             # SBUF partitions
NT = ROWS // P      # 16 row groups per partition
PREDPAD = 128       # bf16 slots reserved for the f32 pred block (256 B)

# kernel configuration knobs (tuned via TimelineSim)
CFG = {
    "k0": 8,          # row groups in the SWDGE-gathered chunk0 (must be even)
    "sq_eng": "act",  # squares on ACT (Square) vs DVE (tensor_tensor mult)
    "swdge_out": True,   # output via kv_writeback prep/trigger vs HWDGE DMA
    "swdge_in": True,    # chunk0 via dma_gather prep/trigger vs HWDGE DMA
}

_NC_CACHE = {}


def _build_nc(cfg=None):
    import concourse.mybir as mybir
    import concourse.tile as tile
    from concourse import bacc

    cfg = dict(CFG if cfg is None else cfg)
    K0 = cfg["k0"]
    K1 = NT - K0
    f32 = mybir.dt.float32
    bf16 = mybir.dt.bfloat16
    i16 = mybir.dt.int16
    i32 = mybir.dt.int32
    Alu = mybir.AluOpType
    Act = mybir.ActivationFunctionType

    nc = bacc.Bacc("TRN2", target_bir_lowering=False, debug=False)

    # chunk0: [pred as f32 bits in PREDPAD bf16 slots | K0 row groups]
    c0_d = nc.dram_tensor("c0", [P, PREDPAD + K0 * D], bf16, kind="ExternalInput")
    c1_d = nc.dram_tensor("c1", [P, K1 * D], bf16, kind="ExternalInput")
    out_d = nc.dram_tensor("out", [C, D], f32, kind="ExternalOutput")

    with tile.TileContext(nc) as tc:
        with (
            tc.tile_pool(name="const", bufs=1) as const,
            tc.tile_pool(name="ps", bufs=1, space="PSUM") as ps,
        ):
            # --- metadata, produced on Pool right after the preamble ---
            idx16 = const.tile([16, 8], i16)  # gather idxs: idx[j%16, j//16]=j
            nc.gpsimd.iota(
                idx16[:], pattern=[[16, 8]], base=0, channel_multiplier=1,
                allow_small_or_imprecise_dtypes=True,
            )
            iot = const.tile([P, C], bf16)  # class ramp 0..C-1
            nc.gpsimd.iota(
                iot[:], pattern=[[1, C]], base=0, channel_multiplier=0,
                allow_small_or_imprecise_dtypes=True,
            )
            ctx = const.tile([P, 1], i32)  # kv_writeback ctx idx = 0
            nc.gpsimd.memset(ctx[:], 0)
            # Dummy rsqrt on ones: forces the act-table pass to load the set
            # containing Abs_reciprocal_sqrt during the input DMA. drs (==1.0)
            # stays live as the multiplier of the final PSUM->SBUF copy.
            onec = const.tile([C, 1], f32)
            nc.gpsimd.memset(onec[:], 1.0)
            drs = const.tile([C, 1], f32)
            nc.scalar.activation(drs[:], onec[:], Act.Abs_reciprocal_sqrt)

            # --- input DMAs ---
            c0t = const.tile([P, 1, PREDPAD + K0 * D], bf16)
            swdge_sems = tc.sems.swdge_block()
            next_lane = iter(range(len(swdge_sems)))
            if cfg["swdge_in"]:
                nc.gpsimd.dma_gather(
                    c0t[:], c0_d[:], idx16[:], 128, 128, PREDPAD + K0 * D,
                    prepare_only=True, sem=swdge_sems[next(next_lane)], queue_num=0,
                )
                nc.gpsimd.trigger_dma(count=None, queue_num=0)
            else:
                nc.sync.dma_start(c0t[:, 0, :], c0_d[:])
            c1t = const.tile([P, K1, D], bf16)
            nc.sync.dma_start(
                c1t[:], c1_d[:].rearrange("p (j d) -> p j d", d=D)
            )

            # output descriptor prep during the input wait
            res = const.tile([C, D], f32)
            if cfg["swdge_out"]:
                nc.gpsimd.kv_writeback(
                    out_d[:].rearrange("c (a b) -> c a b", a=2).unsqueeze(0),
                    res[:].rearrange("c (a b) -> c a b", a=2).unsqueeze(2),
                    ctx[:, 0:1],
                    prepare_only=True, sem=swdge_sems[next(next_lane)], queue_num=0,
                )

            pred_f = c0t[:, 0, 0:32].bitcast(f32)  # [P, 16] f32 labels

            def fch(n):  # raw bf16 features of row group n: [P, D]
                if n < K0:
                    return c0t[:, 0, PREDPAD + n * D : PREDPAD + (n + 1) * D]
                m = n - K0
                return c1t[:, m, :]

            sq = const.tile([P, NT], f32)    # sum of squares per row
            rn = const.tile([P, NT], f32)    # 1/norm per row
            ohw = const.tile([P, NT, C], bf16)  # weighted onehot
            acc = ps.tile([C, D], f32)

            chunks = [list(range(0, K0)), list(range(K0, NT))]
            chunks = [ch for ch in chunks if ch]

            def fv(ci):  # [P, g, D] view of chunk ci's features
                if ci == 0:
                    return c0t[:, 0, PREDPAD:].rearrange("p (j d) -> p j d", d=D)
                return c1t[:, :, :]

            scrs = [const.tile([P, len(ch), D], bf16, name=f"scr{ci}")
                    for ci, ch in enumerate(chunks)]
            # squares (ACT or DVE), then grouped row-sums on DVE
            for ci, ch in enumerate(chunks):
                if cfg["sq_eng"] == "act":
                    nc.scalar.activation(scrs[ci][:], fv(ci), Act.Square)
                else:
                    nc.vector.tensor_tensor(scrs[ci][:], fv(ci), fv(ci), Alu.mult)
            for ci, ch in enumerate(chunks):
                lo, hi = ch[0], ch[-1] + 1
                nc.vector.tensor_reduce(
                    sq[:, lo:hi], scrs[ci][:], axis=mybir.AxisListType.X, op=Alu.add
                )
                nc.scalar.activation(
                    rn[:, lo:hi], sq[:, lo:hi], Act.Abs_reciprocal_sqrt
                )
            for ci, ch in enumerate(chunks):
                for n in ch:
                    nc.vector.tensor_scalar(
                        ohw[:, n, :], iot[:], pred_f[:, n : n + 1],
                        rn[:, n : n + 1], Alu.is_equal, Alu.mult,
                    )
                for n in ch:
                    nc.tensor.matmul(
                        acc[:], ohw[:, n, :], fch(n),
                        start=(n == chunks[0][0]), stop=(n == chunks[-1][-1]),
                    )

            nc.vector.tensor_scalar(res[:], acc[:], drs[:, 0:1], None, Alu.mult)
            if cfg["swdge_out"]:
                nc.gpsimd.trigger_dma(count=None, queue_num=0)
            else:
                nc.sync.dma_start(out_d[:], res[:])

    nc.compile()
    return nc


def _get_nc(cfg=None):
    key = "nc" if cfg is None else str(sorted(cfg.items()))
    if key not in _NC_CACHE:
        _NC_CACHE[key] = _build_nc(cfg)
    return _NC_CACHE[key]


def _make_in_maps(feature, pred, cfg=None):
    import ml_dtypes

    cfg = CFG if cfg is None else cfg
    K0 = cfg["k0"]
    K1 = NT - K0
    bf16 = ml_dtypes.bfloat16
    feature = np.asarray(feature).astype(bf16)
    pred_f = np.asarray(pred).astype(np.float32)
    in_maps = []
    for c in range(NCORES):
        F = feature[c * ROWS : (c + 1) * ROWS].reshape(P, NT, D)
        pf = np.ascontiguousarray(pred_f[c * ROWS : (c + 1) * ROWS].reshape(P, NT))
        c0 = np.zeros((P, PREDPAD + K0 * D), bf16)
        c0[:, 0:32] = pf.view(np.uint16).view(bf16)  # f32 bits in bf16 slots
        c0[:, PREDPAD:] = F[:, :K0].reshape(P, K0 * D)
        c1 = np.ascontiguousarray(F[:, K0:].reshape(P, K1 * D))
        in_maps.append({"c0": c0, "c1": c1})
    return in_maps


def _finish(partials, pred):
    """Combine per-core partial segment sums into the scalar loss."""
    pred_i = np.asarray(pred).astype(np.int64)
    S = np.zeros((C, D), np.float64)
    for p in partials:
        S += p.astype(np.float64)
    counts = np.bincount(pred_i, minlength=C).astype(np.float64)
    cls_pair_sum = 0.5 * ((S * S).sum(axis=1) - counts)
    pair_counts = counts * (counts - 1.0) * 0.5
    avg = np.where(pair_counts > 0, cls_pair_sum / np.maximum(pair_counts, 1.0), 0.0)
    n_unique = float((counts > 0).sum())
    loss = 1.0 - avg.sum() / n_unique
    return np.float32(loss)


def _unscramble(O):
    # kv_writeback walks input d_head lines dho-major but output lines
    # dhi-major: returned line l (of [128,32]) holds S[l % 64, (l//64)*32:+32].
    L = np.asarray(O).reshape(2 * C, D // 2)
    return np.concatenate([L[:C], L[C:]], axis=1)


def _run(feature, pred, trace=False, cfg=None, **spmd_kwargs):
    from concourse.bass_utils import run_bass_kernel_spmd

    nc = _get_nc(cfg)
    in_maps = _make_in_maps(feature, pred, cfg)
    res = run_bass_kernel_spmd(
        nc, in_maps, core_ids=list(range(NCORES)), trace=trace, **spmd_kwargs
    )
    swout = (CFG if cfg is None else cfg)["swdge_out"]
    partials = [_unscramble(r["out"]) if swout else r["out"] for r in res.results]
    return _finish(partials, pred), res


def kernel(feature, pred, num_classes):
    assert int(num_classes) == C
    loss, _ = _run(feature, pred, trace=False)
    return loss
